# revision 43
# baseline (speedup 1.0000x reference)
"""Segment-sum (jax.ops.segment_sum(H, X_node, num_segments=V)) on 8 trn2
NeuronCores.

Strategy
--------
Host-side sharding (sorted deal): segments sorted by token count are
dealt in groups of 128 similar-count segments to (core, block, slot);
each core produces a disjoint set of output rows, so no device reduce is
needed and the host applies the inverse permutation on gather.

Because slot counts within a block are nearly uniform, ~98% of tiles are
identity-diagonals: the r-th token of slot s sits at partition s of tile
r, and the tensor engine accumulates it with a CONSTANT identity lhsT —
no per-tile one-hot build.  Only per-slot overflow tokens go to dense
tiles whose one-hot is built on the vector engine (tensor_scalar
is_equal, DVE 4x mode).  Per-local-block (J_diag, T_overflow) budgets are
shared by all 8 cores (one static program, ~0.6% padding).

H is shipped as a bf16 hi/lo split (same bytes as f32): the matmul runs
at bf16 rate with exact products and fp32 PSUM accumulation, giving
~2.5e-6 l2 relative error.  Block psums flush hi+lo into an SBUF stage
streamed out in quarters.

Cost model (validated on HW within ~5-10%): ~201us/core — at the ~196us
DMA floor for the 63MB/core streamed (pure-H f32 roofline ~180us).
"""

import math
import sys

sys.path.insert(0, "/opt/trn_rl_repo")

import numpy as np
import ml_dtypes

import concourse.bacc as bacc
import concourse.mybir as mybir
import concourse.tile as tile
from concourse.bass_utils import run_bass_kernel_spmd

P = 128          # partitions / tokens per tile / segments per block
D = 64           # feature dim
V = 100000       # number of segments
NCORES = 8
NB = 98          # blocks per core (8*98*128 = 100352 >= V)
SLICE = NB * P   # segments per core
TC = 64          # tiles per DMA chunk

_BUILD_CACHE: dict = {}


OH_MODE = "tsp"  # tsp: per-tile tensor_scalar (DVE 4x mode)
                 # ttN: N-tile-batched tensor_tensor (1x mode, fewer PE waits)
PRECISION = "e3m4"   # split: bf16 hi+lo pair (~2.5e-6 l2 err, 4B/elem)
                     # bf16: single bf16 (~1e-3 l2 err, 2B/elem, ~1.6x faster)
                     # e3m4: fp8 E3M4 (~1.33e-2 l2 err, 1B/elem)
E3_SCALE = 2.0       # pre-scale for e3m4 (undone exactly on host gather)
GROUP = 1      # diag tiles per matmul (pair -> [P,2D] psum, halves PE
               # instruction count; flush folds the halves)
ILEAVE = 4     # blocks interleaved round-robin in the tile stream, so
               # consecutive matmuls accumulate into DIFFERENT psum banks
               # (same-bank back-to-back accumulates stall ~53-79ns on the
               # PE->PSUM->PE read-after-write; interleaved they pipeline
               # at the ~25ns issue rate)
OP_BUFS = 24   # onehot tiles in flight (DVE -> PE pipeline depth)
OH_ACT_EVERY = 0  # 0=off; N>0: every Nth tile's onehot via 2-op ACT chain
OH_DIAG = 0    # first J tiles of each block are identity-diagonal (no DVE)
HP_BUFS = 8    # H chunk buffers
FLUSH_ENGINE = "vector"  # vector | scalar (PSUM hi-half copy engine)


def _order(budgets: tuple):
    """Interleaved tile emission order: blocks in groups of ILEAVE emit
    their tiles round-robin.  Returns (tsched, old_idx): tsched[i] =
    (block, is_diag) of the i-th emitted tile; old_idx[i] = its index in
    the plain block-sequential order (for the host's token placement)."""
    nb = len(budgets)
    off = [0]
    for J, T in budgets:
        off.append(off[-1] + J + T)
    tsched, old_idx = [], []
    for b0 in range(0, nb, ILEAVE):
        blocks = list(range(b0, min(b0 + ILEAVE, nb)))
        pos = {b: 0 for b in blocks}
        left = {b: budgets[b][0] + budgets[b][1] for b in blocks}
        while any(left[b] for b in blocks):
            for b in blocks:
                if left[b]:
                    j = pos[b]
                    tsched.append((b, j < budgets[b][0]))
                    old_idx.append(off[b] + j)
                    pos[b] += 1
                    left[b] -= 1
    return tsched, old_idx


def _build(nb: int, budgets: tuple, tc: int, nchunks: int,
           variant: str = "full"):
    """Static SPMD program. budgets: per-local-block (J_diag, T_overflow)
    tuples, identical across cores; tiles j<J use a constant identity lhsT
    (no one-hot build), the rest build a one-hot on DVE."""
    key = (nb, budgets, tc, nchunks, variant, OH_MODE, PRECISION,
           OP_BUFS, HP_BUFS, GROUP)
    if key in _BUILD_CACHE:
        return _BUILD_CACHE[key]
    reps = 1
    hw_loop = 0
    small_out = False
    if variant.startswith("ts"):
        reps = int(variant[2:])
        small_out = True
        variant = "full"
    elif variant.startswith("dma") and variant[3:].isdigit():
        reps = int(variant[3:])
        small_out = True
        variant = "dmaonly"
    elif variant.startswith("fi"):
        # hardware For_i loop around the whole body: exec time ~ N * body,
        # NEFF stays small -> pipelined wall/exec ~= N * per-rep time
        hw_loop = int(variant[2:])
        small_out = True
        variant = "full"
    elif variant.startswith("fo"):
        # For_i loop with the full output store inside each iteration
        hw_loop = int(variant[2:])
        variant = "full"
    elif variant.startswith("fd"):
        hw_loop = int(variant[2:])
        small_out = True
        variant = "dmaonly"
    elif variant.startswith("pb"):
        # pure-PE microbench: ntiles back-to-back matmuls on one resident
        # chunk, no DMA/DVE in the loop -> measures the real PE col rate
        hw_loop = int(variant[2:])
        small_out = True
        variant = "peonly"
    elif variant.startswith("pa"):
        # like pb but PSUM-accumulating chains of 20 (the real kernel's
        # start=False pattern) -> isolates the accumulate-matmul rate
        hw_loop = int(variant[2:])
        small_out = True
        variant = "peacc"
    # flat tile schedule: (block, is_diag), block-interleaved
    tsched, _ = _order(budgets)
    ntiles = len(tsched)
    assert nchunks * tc >= ntiles
    blk_first, blk_last = {}, {}
    for i, (b, _) in enumerate(tsched):
        blk_first.setdefault(b, i)
        blk_last[b] = i
    grp = GROUP if (PRECISION != "split" and variant != "halfpe") else 1
    # grouped matmul schedule per chunk: consecutive diag-tile PAIRS of a
    # block (within one chunk) share one matmul into a [P, 2D] psum: the
    # even tile accumulates in cols [0,D), the odd tile in [D,2D); the
    # flush folds the halves.  Singletons (chunk straddle, odd J, one-hot
    # overflow) write cols [0,D) -- except the one that must initialize
    # the upper half (start=True at col D) when a block opens unpaired.
    # entries: (k, w, blk, diag, col, start, stop, alloc)
    gsched = [[] for _ in range(nchunks)]
    half1_ok = set()
    g = 0
    while g < ntiles:
        b, diag = tsched[g]
        ch, k = divmod(g, tc)
        w = 1
        if (grp == 2 and diag and k + 1 < tc and g + 1 < ntiles
                and tsched[g + 1] == (b, True)):
            w = 2
        alloc = g == blk_first[b]
        if w == 2:
            col, start = 0, alloc
            half1_ok.add(b)
        elif grp == 2 and diag and b not in half1_ok and not alloc:
            col, start = 1, True     # initialize upper psum half
            half1_ok.add(b)
        else:
            col, start = 0, alloc
        stop = g + w - 1 == blk_last[b]
        gsched[ch].append((k, w, b, diag, col, start, stop, alloc))
        g += w
    pw = 2 * D if (PRECISION == "split" or grp == 2) else D
    ew = 2 * D if PRECISION == "split" else D  # elems per token in h stream
    hdt = (mybir.dt.float8e3 if PRECISION == "e3m4"
           else mybir.dt.bfloat16)
    # number the overflow (one-hot) tiles: slot-ids are shipped only for
    # these columns
    oix_of = {}
    for i, (b, dg) in enumerate(tsched):
        if not dg:
            oix_of[i] = len(oix_of)
    novf = max(1, len(oix_of))
    # the output leaves as bf16 (cast on the Pool-engine DMA); the host
    # widens back.  ~0.17% elementwise rounding, invisible next to the
    # e3m4/bf16 stream encoding error.
    odt = mybir.dt.float32 if PRECISION == "split" else mybir.dt.bfloat16
    nc = bacc.Bacc("TRN2")
    hin = nc.dram_tensor("h", [nchunks, P, tc * ew], hdt,
                         kind="ExternalInput")
    lin = nc.dram_tensor("lo", [P, novf], mybir.dt.float32,
                         kind="ExternalInput")
    iin = nc.dram_tensor("iota", [P, P], mybir.dt.bfloat16,
                         kind="ExternalInput")
    idn = nc.dram_tensor("ident", [P, P], mybir.dt.bfloat16,
                         kind="ExternalInput")
    out = nc.dram_tensor("out", [P, D if small_out else nb * D],
                         odt, kind="ExternalOutput")
    # the DVE flush copy writes ostage already in the output dtype (free
    # conversion), so the store is a plain same-dtype SP-engine DMA
    ostore = nc.sync.dma_start

    with tile.TileContext(nc) as tc_ctx:
        with (
            tc_ctx.tile_pool(name="hp", bufs=HP_BUFS) as hp,
            tc_ctx.tile_pool(name="op", bufs=OP_BUFS) as op,
            tc_ctx.tile_pool(name="pp", bufs=8, space="PSUM") as pp,
            tc_ctx.tile_pool(name="cp", bufs=1) as cp,
        ):
            iota = cp.tile([P, P], mybir.dt.bfloat16, tag="iota")
            nc.sync.dma_start(iota[:], iin[:])
            const_oh = cp.tile([P, P], mybir.dt.bfloat16, tag="constoh")
            nc.sync.dma_start(const_oh[:], idn[:])
            ostage = cp.tile([P, nb * D], odt, tag="ostage")
            if variant in ("dmaonly", "nope"):
                nc.gpsimd.memset(ostage[:], 0.0)
            lall = cp.tile([P, novf], mybir.dt.float32, tag="lall")
            nc.sync.dma_start(lall[:], lin[:])
            import contextlib
            psum, psums = None, {}
            if variant in ("peonly", "peacc"):
                hres = cp.tile([P, tc * ew], hdt, tag="hres")
                nc.sync.dma_start(hres[:], hin[0])
                CH = 20
                with tc_ctx.For_i(0, hw_loop, 1):
                    for i in range(ntiles):
                        acc = variant == "peacc"
                        st = (i % CH == 0) if acc else True
                        sp = (i % CH == CH - 1 or i == ntiles - 1) \
                            if acc else True
                        if st:
                            psum = pp.tile([P, pw], mybir.dt.float32)
                        nc.tensor.matmul(
                            psum[:, :D],
                            lhsT=const_oh[:],
                            rhs=hres[:, (i % tc) * ew:(i % tc) * ew + D],
                            start=st,
                            stop=sp,
                            skip_group_check=True,
                        )
                nc.gpsimd.memset(ostage[:, :D], 0.0)
            loop_cm = (tc_ctx.For_i(0, hw_loop, 1)
                       if hw_loop and variant != "peonly"
                       else contextlib.nullcontext())
            with loop_cm:
              for _rep, ch in (((r, c) for r in range(reps)
                                for c in range(nchunks))
                               if variant != "peonly" else ()):
                htile = hp.tile([P, tc * ew], hdt)
                nc.sync.dma_start(htile[:], hin[ch])
                if variant == "dmaonly":
                    continue
                for (k, w, b, diag, col, start, stop, alloc) in gsched[ch]:
                    g = ch * tc + k
                    ohtile = None
                    if variant != "nodve" and not diag:
                        # tensor_scalar: iota row packed bf16 (in0), slot id
                        # as per-partition scalar -> DVE 4x_2p perf mode
                        ohtile = op.tile([P, P], mybir.dt.bfloat16)
                        nc.vector.tensor_scalar(
                            out=ohtile[:],
                            in0=iota[:],
                            scalar1=lall[:, oix_of[g]:oix_of[g] + 1],
                            scalar2=None,
                            op0=mybir.AluOpType.is_equal,
                        )
                    if variant == "nope":
                        continue
                    oh = (const_oh[:] if (variant == "nodve" or diag)
                          else ohtile[:])
                    nw = D if variant == "halfpe" else w * ew
                    if alloc:
                        psums[b] = pp.tile([P, pw], mybir.dt.float32,
                                           name="ps")
                    psum = psums[b]
                    nc.tensor.matmul(
                        psum[:, col * D:col * D + nw],
                        lhsT=oh,
                        rhs=htile[:, k * ew:k * ew + nw],
                        start=start,
                        stop=stop,
                        skip_group_check=True,
                    )
                    if stop:
                        # DVE may read only one PSUM operand per instruction
                        nc.vector.tensor_copy(
                            out=ostage[:, b * D:(b + 1) * D],
                            in_=psum[:, :D],
                        )
                        if pw == 2 * D:
                            nc.vector.tensor_add(
                                out=ostage[:, b * D:(b + 1) * D],
                                in0=ostage[:, b * D:(b + 1) * D],
                                in1=psum[:, D:2 * D],
                            )
                        # stream completed quarters of ostage out early so
                        # the final store overlaps compute
                        if not small_out and _rep == reps - 1:
                            q = nb // 4
                            if b + 1 in (q, 2 * q, 3 * q):
                                s = (b + 1 - q) * D
                                ostore(
                                    out[:, s:(b + 1) * D],
                                    ostage[:, s:(b + 1) * D])
              if hw_loop:
                if small_out:
                    ostore(out[:], ostage[:, :D])
                else:
                    q = nb // 4
                    ostore(out[:, 3 * q * D:], ostage[:, 3 * q * D:])
            if not hw_loop:
                if small_out:
                    ostore(out[:], ostage[:, :D])
                else:
                    q = nb // 4
                    ostore(out[:, 3 * q * D:], ostage[:, 3 * q * D:])
    nc.finalize()
    _BUILD_CACHE[key] = nc
    return nc


def _host_prep(H: np.ndarray, seg: np.ndarray, ncores: int, nb: int,
               tc: int):
    """Sorted-deal sharding: segments sorted by count desc are dealt in
    groups of 128 similar-count segments to (core = g % ncores,
    local block = g // ncores, slot = position).  Within a block the slot
    counts are nearly uniform, so most tiles are identity-diagonals (the
    r-th token of slot s at partition s of tile r) needing no one-hot
    build; per-slot overflow beyond each block's diagonal depth J goes to
    dense one-hot tiles.  Per-local-block budgets (J, T_overflow) are
    shared by all cores (one static program)."""
    n, d = H.shape
    v = ncores * nb * P
    nblocks = ncores * nb
    cnt_seg = np.bincount(seg, minlength=v)
    order_seg = np.argsort(-cnt_seg, kind="stable")
    # seg order_seg[i]: group g = i // P, slot = i % P
    g_of = np.arange(v) // P
    blk_of = np.empty(v, np.int32)
    slot_of = np.empty(v, np.int32)
    blk_of[order_seg] = ((g_of % ncores) * nb + g_of // ncores).astype(np.int32)
    slot_of[order_seg] = (np.arange(v) % P).astype(np.int32)

    # per-(block, slot) counts and per-local-block budgets
    per_bs = np.zeros((nblocks, P), np.int64)
    per_bs[blk_of, slot_of] = cnt_seg
    budgets = []
    for lb in range(nb):
        M = per_bs[[c * nb + lb for c in range(ncores)]]
        best = None
        for J in range(int(M.min()), int(M.max()) + 1):
            ovf = int(np.maximum(M - J, 0).sum(axis=1).max())
            tov = -(-ovf // P)
            # <= : on equal totals prefer larger J (fewer one-hot tiles)
            if best is None or J + tov <= best[0] + best[1]:
                best = (J, tov)
        budgets.append(best)
    budgets = tuple(budgets)
    off = np.zeros(nb + 1, np.int64)
    np.cumsum(np.array([j + t for j, t in budgets]), out=off[1:])
    ntiles = int(off[-1])
    nchunks = int(math.ceil(ntiles / tc))
    rows_pad = nchunks * tc * P

    # token destinations
    key = blk_of[seg].astype(np.int64) * P + slot_of[seg]
    order = np.argsort(key, kind="stable")
    skey = key[order]
    cnt_key = np.bincount(skey, minlength=nblocks * P)
    starts_key = np.zeros(nblocks * P + 1, np.int64)
    np.cumsum(cnt_key, out=starts_key[1:])
    r = np.arange(n) - starts_key[skey]            # rank within (block, slot)
    sblk = skey // P
    cnt_blk = np.bincount(sblk, minlength=nblocks)
    starts_blk = np.zeros(nblocks + 1, np.int64)
    np.cumsum(cnt_blk, out=starts_blk[1:])
    lbs = sblk % nb
    core = sblk // nb
    J_of = np.array([b[0] for b in budgets], np.int64)[lbs]
    is_ovf = r >= J_of
    co = np.cumsum(is_ovf)
    coz = np.concatenate([[0], co])[starts_blk[sblk]]  # ovf before block
    q = co - 1 - coz                                   # ovf index in block
    tile_idx = np.where(is_ovf, off[lbs] + J_of + q // P, off[lbs] + r)
    # remap to the block-interleaved emission order used by the device
    tsched_new, old_idx = _order(budgets)
    perm = np.empty(ntiles, np.int64)
    perm[np.array(old_idx, np.int64)] = np.arange(ntiles)
    tile_idx = perm[tile_idx]
    part = np.where(is_ovf, q % P, skey % P)
    dstrow = tile_idx * P + part
    lo_val = (skey % P).astype(np.float32)

    iota = np.broadcast_to(np.arange(P), (P, P)).astype(ml_dtypes.bfloat16)
    ident = np.eye(P, dtype=ml_dtypes.bfloat16)
    in_maps = []
    for c in range(ncores):
        sel = core == c
        dst_c = dstrow[sel]
        assert dst_c.max() < rows_pad
        assert np.unique(dst_c).size == dst_c.size, "dst collision"
        rows = np.zeros((rows_pad, d), np.float32)
        lo = np.zeros(rows_pad, np.float32)
        rows[dst_c] = H[order[sel]]
        lo[dst_c] = lo_val[sel]
        if PRECISION == "e3m4":
            hl = np.clip(rows * E3_SCALE, -15.5, 15.5).astype(
                ml_dtypes.float8_e3m4).reshape(rows_pad // P, P, d)
            ew = d
        else:
            hi = rows.astype(ml_dtypes.bfloat16)
            if PRECISION == "split":
                lo_res = (rows - hi.astype(np.float32)).astype(
                    ml_dtypes.bfloat16)
                hl = np.concatenate(
                    [hi.reshape(rows_pad // P, P, d),
                     lo_res.reshape(rows_pad // P, P, d)],
                    axis=-1,
                )  # [ntiles_pad, P, 2D]
                ew = 2 * d
            else:
                hl = hi.reshape(rows_pad // P, P, d)
                ew = d
        hdev = np.ascontiguousarray(
            hl.reshape(nchunks, tc, P, ew).transpose(0, 2, 1, 3)
        ).reshape(nchunks, P, tc * ew)
        # slot-ids only for the overflow (one-hot) tiles, in schedule order
        ovf_g = np.array([i for i, (_b2, dg2) in enumerate(tsched_new)
                          if not dg2], dtype=np.int64)
        if ovf_g.size:
            lodev = np.ascontiguousarray(
                lo.reshape(nchunks * tc, P)[ovf_g].T)
        else:
            lodev = np.zeros((P, 1), np.float32)
        in_maps.append({"h": hdev, "lo": lodev, "iota": iota,
                        "ident": ident})
    outperm = blk_of.astype(np.int64) * P + slot_of  # seg -> output slot
    return in_maps, budgets, nchunks, outperm


def _unshard(results, ncores: int, nb: int, outperm: np.ndarray) -> np.ndarray:
    full = np.empty((ncores * nb * P, D), np.float32)
    for c in range(ncores):
        o = np.asarray(results[c]["out"]).astype(np.float32).reshape(P, nb, D)
        full[c * nb * P:(c + 1) * nb * P] = (
            o.transpose(1, 0, 2).reshape(nb * P, D)
        )
    if PRECISION == "e3m4":
        full *= 1.0 / E3_SCALE
    return full[outperm]


def _run(H, X_node, trace=False, trace_kwargs=None):
    H = np.ascontiguousarray(np.asarray(H, dtype=np.float32))
    seg = np.asarray(X_node).astype(np.int64)
    in_maps, budgets, nchunks, outperm = _host_prep(H, seg, NCORES, NB, TC)
    nc = _build(NB, budgets, TC, nchunks)
    kwargs = {}
    if trace:
        kwargs = dict(trace=True, trace_cores=list(range(NCORES)),
                      stitch_traces=False)
        if trace_kwargs:
            kwargs.update(trace_kwargs)
    res = run_bass_kernel_spmd(nc, in_maps, core_ids=list(range(NCORES)),
                               **kwargs)
    out = _unshard(res.results, NCORES, NB, outperm[:V])
    return out, res


def kernel(H, X_node) -> np.ndarray:
    out, _ = _run(H, X_node, trace=False)
    return out


if __name__ == "__main__":
    # tiny smoke test on hardware (all 8 cores, small V')
    rng = np.random.default_rng(0)
    n_small, v_small, nb_small, tc_small = 6000, NCORES * 2 * P, 2, 4
    Hs = rng.standard_normal((n_small, D)).astype(np.float32)
    segs = rng.integers(0, v_small, size=n_small).astype(np.int64)
    in_maps, budgets, nchunks, outperm = _host_prep(Hs, segs, NCORES, nb_small,
                                                    tc_small)
    nc = _build(nb_small, budgets, tc_small, nchunks)
    res = run_bass_kernel_spmd(nc, in_maps, core_ids=list(range(NCORES)))
    got = _unshard(res.results, NCORES, nb_small, outperm[:v_small])
    # compare against the segment-sum of the DECODED codes: isolates a
    # HW-vs-ml_dtypes encoding mismatch (e.g. fp8 subnormal flush) from
    # the intended quantization error
    if PRECISION == "e3m4":
        Hq = (np.clip(Hs * E3_SCALE, -15.5, 15.5)
              .astype(ml_dtypes.float8_e3m4).astype(np.float32) / E3_SCALE)
    elif PRECISION == "bf16":
        Hq = Hs.astype(ml_dtypes.bfloat16).astype(np.float32)
    else:
        Hq = Hs
    exp = np.zeros((v_small, D), np.float32)
    np.add.at(exp, segs, Hq)
    if PRECISION != "split":   # device stores the output as bf16
        exp = (exp * E3_SCALE if PRECISION == "e3m4" else exp)
        exp = exp.astype(ml_dtypes.bfloat16).astype(np.float32)
        exp = (exp / E3_SCALE if PRECISION == "e3m4" else exp)
    err = np.abs(got - exp).max() / max(1e-9, np.abs(exp).max())
    print(f"smoke: ntiles={sum(j+t for j,t in budgets)} nchunks={nchunks} max-rel-err={err:.3e}")
    assert err < 1e-4, "smoke test failed"
    print("SMOKE PASS")



# revision 45
# speedup vs baseline: 1.6739x; 1.6739x over previous
"""Segment-sum (jax.ops.segment_sum(H, X_node, num_segments=V)) on 8 trn2
NeuronCores.

Strategy
--------
Host-side sharding (sorted deal): segments sorted by token count are
dealt in groups of 128 similar-count segments to (core, block, slot);
each core produces a disjoint set of output rows, so no device reduce is
needed and the host applies the inverse permutation on gather.

Because slot counts within a block are nearly uniform, ~99.7% of tiles
are identity-diagonals: the r-th token of slot s sits at partition s of
tile r, and the tensor engine accumulates it with a CONSTANT identity
lhsT — no per-tile one-hot build.  Only per-slot overflow tokens go to
dense tiles whose one-hot is built on the vector engine (tensor_scalar
is_equal); slot-ids are shipped only for those few columns.  Per-block
(J_diag, T_overflow) budgets are shared by all 8 cores (one static
program, ~1% padding).

H is shipped as fp8 E3M4 (1 byte/elem, x2 pre-scale undone exactly on
the host gather): products against the 0/1 lhsT are exact and accumulate
in fp32 PSUM, so the only error is the E3M4 encoding — 1.34e-2 l2
(verified bit-exact vs ml_dtypes on HW, subnormals included) against the
2e-2 gate.  The output leaves the device as bf16 (the DVE flush writes
the staging buffer pre-converted; +0.17%/elem, invisible at this l2).
Earlier encodings, kept selectable via PRECISION: bf16 hi/lo split 4B
(2.5e-6 l2, ~210us) and single bf16 2B (1.7e-3 l2, ~137us).

Measured on HW (slope of a For_i-looped full body, 16 vs 64 iterations,
all 8 cores streaming): ~103us/core.  The same-method DMA-only floor is
~39-49us; the binding resource is the PE accumulate stream: accumulating
(start=False) matmuls sustain ~0.83ns/col — 2x the 0.42ns/col of
non-accumulating ones (pb/pa microbench variants) — so 1984 tiles x 64
cols pins the PE near ~105us.  Interleaving psum banks (ILEAVE) does not
lift the accumulate rate, and DVE/ACT pre-adds cannot absorb enough
columns to beat it (fp8 runs at 1x DVE mode).
"""

import math
import sys

sys.path.insert(0, "/opt/trn_rl_repo")

import numpy as np
import ml_dtypes

import concourse.bacc as bacc
import concourse.mybir as mybir
import concourse.tile as tile
from concourse.bass_utils import run_bass_kernel_spmd

P = 128          # partitions / tokens per tile / segments per block
D = 64           # feature dim
V = 100000       # number of segments
NCORES = 8
NB = 98          # blocks per core (8*98*128 = 100352 >= V)
SLICE = NB * P   # segments per core
TC = 32          # tiles per DMA chunk

_BUILD_CACHE: dict = {}


OH_MODE = "tsp"  # tsp: per-tile tensor_scalar (DVE 4x mode)
                 # ttN: N-tile-batched tensor_tensor (1x mode, fewer PE waits)
PRECISION = "e3m4"   # split: bf16 hi+lo pair (~2.5e-6 l2 err, 4B/elem)
                     # bf16: single bf16 (~1e-3 l2 err, 2B/elem, ~1.6x faster)
                     # e3m4: fp8 E3M4 (~1.33e-2 l2 err, 1B/elem)
E3_SCALE = 2.0       # pre-scale for e3m4 (undone exactly on host gather)
GROUP = 1      # diag tiles per matmul (pair -> [P,2D] psum, halves PE
               # instruction count; flush folds the halves)
ILEAVE = 1     # blocks interleaved round-robin in the tile stream, so
               # consecutive matmuls accumulate into DIFFERENT psum banks
               # (same-bank back-to-back accumulates stall ~53-79ns on the
               # PE->PSUM->PE read-after-write; interleaved they pipeline
               # at the ~25ns issue rate)
OP_BUFS = 24   # onehot tiles in flight (DVE -> PE pipeline depth)
OH_ACT_EVERY = 0  # 0=off; N>0: every Nth tile's onehot via 2-op ACT chain
OH_DIAG = 0    # first J tiles of each block are identity-diagonal (no DVE)
HP_BUFS = 8    # H chunk buffers
FLUSH_ENGINE = "vector"  # vector | scalar (PSUM hi-half copy engine)


def _order(budgets: tuple):
    """Interleaved tile emission order: blocks in groups of ILEAVE emit
    their tiles round-robin.  Returns (tsched, old_idx): tsched[i] =
    (block, is_diag) of the i-th emitted tile; old_idx[i] = its index in
    the plain block-sequential order (for the host's token placement)."""
    nb = len(budgets)
    off = [0]
    for J, T in budgets:
        off.append(off[-1] + J + T)
    tsched, old_idx = [], []
    for b0 in range(0, nb, ILEAVE):
        blocks = list(range(b0, min(b0 + ILEAVE, nb)))
        pos = {b: 0 for b in blocks}
        left = {b: budgets[b][0] + budgets[b][1] for b in blocks}
        while any(left[b] for b in blocks):
            for b in blocks:
                if left[b]:
                    j = pos[b]
                    tsched.append((b, j < budgets[b][0]))
                    old_idx.append(off[b] + j)
                    pos[b] += 1
                    left[b] -= 1
    return tsched, old_idx


def _build(nb: int, budgets: tuple, tc: int, nchunks: int,
           variant: str = "full"):
    """Static SPMD program. budgets: per-local-block (J_diag, T_overflow)
    tuples, identical across cores; tiles j<J use a constant identity lhsT
    (no one-hot build), the rest build a one-hot on DVE."""
    key = (nb, budgets, tc, nchunks, variant, OH_MODE, PRECISION,
           OP_BUFS, HP_BUFS, GROUP, ILEAVE)
    if key in _BUILD_CACHE:
        return _BUILD_CACHE[key]
    reps = 1
    hw_loop = 0
    small_out = False
    if variant.startswith("ts"):
        reps = int(variant[2:])
        small_out = True
        variant = "full"
    elif variant.startswith("dma") and variant[3:].isdigit():
        reps = int(variant[3:])
        small_out = True
        variant = "dmaonly"
    elif variant.startswith("fi"):
        # hardware For_i loop around the whole body: exec time ~ N * body,
        # NEFF stays small -> pipelined wall/exec ~= N * per-rep time
        hw_loop = int(variant[2:])
        small_out = True
        variant = "full"
    elif variant.startswith("fo"):
        # For_i loop with the full output store inside each iteration
        hw_loop = int(variant[2:])
        variant = "full"
    elif variant.startswith("fd"):
        hw_loop = int(variant[2:])
        small_out = True
        variant = "dmaonly"
    elif variant.startswith("pb"):
        # pure-PE microbench: ntiles back-to-back matmuls on one resident
        # chunk, no DMA/DVE in the loop -> measures the real PE col rate
        hw_loop = int(variant[2:])
        small_out = True
        variant = "peonly"
    elif variant.startswith("pa"):
        # like pb but PSUM-accumulating chains of 20 (the real kernel's
        # start=False pattern) -> isolates the accumulate-matmul rate
        hw_loop = int(variant[2:])
        small_out = True
        variant = "peacc"
    # flat tile schedule: (block, is_diag), block-interleaved
    tsched, _ = _order(budgets)
    ntiles = len(tsched)
    assert nchunks * tc >= ntiles
    blk_first, blk_last = {}, {}
    for i, (b, _) in enumerate(tsched):
        blk_first.setdefault(b, i)
        blk_last[b] = i
    grp = GROUP if (PRECISION != "split" and variant != "halfpe") else 1
    # grouped matmul schedule per chunk: consecutive diag-tile PAIRS of a
    # block (within one chunk) share one matmul into a [P, 2D] psum: the
    # even tile accumulates in cols [0,D), the odd tile in [D,2D); the
    # flush folds the halves.  Singletons (chunk straddle, odd J, one-hot
    # overflow) write cols [0,D) -- except the one that must initialize
    # the upper half (start=True at col D) when a block opens unpaired.
    # entries: (k, w, blk, diag, col, start, stop, alloc)
    gsched = [[] for _ in range(nchunks)]
    half1_ok = set()
    g = 0
    while g < ntiles:
        b, diag = tsched[g]
        ch, k = divmod(g, tc)
        w = 1
        if (grp == 2 and diag and k + 1 < tc and g + 1 < ntiles
                and tsched[g + 1] == (b, True)):
            w = 2
        alloc = g == blk_first[b]
        if w == 2:
            col, start = 0, alloc
            half1_ok.add(b)
        elif grp == 2 and diag and b not in half1_ok and not alloc:
            col, start = 1, True     # initialize upper psum half
            half1_ok.add(b)
        else:
            col, start = 0, alloc
        stop = g + w - 1 == blk_last[b]
        gsched[ch].append((k, w, b, diag, col, start, stop, alloc))
        g += w
    pw = 2 * D if (PRECISION == "split" or grp == 2) else D
    ew = 2 * D if PRECISION == "split" else D  # elems per token in h stream
    hdt = (mybir.dt.float8e3 if PRECISION == "e3m4"
           else mybir.dt.bfloat16)
    # number the overflow (one-hot) tiles: slot-ids are shipped only for
    # these columns
    oix_of = {}
    for i, (b, dg) in enumerate(tsched):
        if not dg:
            oix_of[i] = len(oix_of)
    novf = max(1, len(oix_of))
    # the output leaves as bf16 (cast on the Pool-engine DMA); the host
    # widens back.  ~0.17% elementwise rounding, invisible next to the
    # e3m4/bf16 stream encoding error.
    odt = mybir.dt.float32 if PRECISION == "split" else mybir.dt.bfloat16
    nc = bacc.Bacc("TRN2")
    hin = nc.dram_tensor("h", [nchunks, P, tc * ew], hdt,
                         kind="ExternalInput")
    lin = nc.dram_tensor("lo", [P, novf], mybir.dt.float32,
                         kind="ExternalInput")
    iin = nc.dram_tensor("iota", [P, P], mybir.dt.bfloat16,
                         kind="ExternalInput")
    idn = nc.dram_tensor("ident", [P, P], mybir.dt.bfloat16,
                         kind="ExternalInput")
    out = nc.dram_tensor("out", [P, D if small_out else nb * D],
                         odt, kind="ExternalOutput")
    # the DVE flush copy writes ostage already in the output dtype (free
    # conversion), so the store is a plain same-dtype SP-engine DMA
    ostore = nc.sync.dma_start

    with tile.TileContext(nc) as tc_ctx:
        with (
            tc_ctx.tile_pool(name="hp", bufs=HP_BUFS) as hp,
            tc_ctx.tile_pool(name="op", bufs=OP_BUFS) as op,
            tc_ctx.tile_pool(name="pp", bufs=8, space="PSUM") as pp,
            tc_ctx.tile_pool(name="cp", bufs=1) as cp,
        ):
            iota = cp.tile([P, P], mybir.dt.bfloat16, tag="iota")
            nc.sync.dma_start(iota[:], iin[:])
            const_oh = cp.tile([P, P], mybir.dt.bfloat16, tag="constoh")
            nc.sync.dma_start(const_oh[:], idn[:])
            ostage = cp.tile([P, nb * D], odt, tag="ostage")
            if variant in ("dmaonly", "nope"):
                nc.gpsimd.memset(ostage[:], 0.0)
            lall = cp.tile([P, novf], mybir.dt.float32, tag="lall")
            nc.sync.dma_start(lall[:], lin[:])
            import contextlib
            psum, psums = None, {}
            if variant in ("peonly", "peacc"):
                hres = cp.tile([P, tc * ew], hdt, tag="hres")
                nc.sync.dma_start(hres[:], hin[0])
                CH = 20
                with tc_ctx.For_i(0, hw_loop, 1):
                    for i in range(ntiles):
                        acc = variant == "peacc"
                        st = (i % CH == 0) if acc else True
                        sp = (i % CH == CH - 1 or i == ntiles - 1) \
                            if acc else True
                        if st:
                            psum = pp.tile([P, pw], mybir.dt.float32)
                        nc.tensor.matmul(
                            psum[:, :D],
                            lhsT=const_oh[:],
                            rhs=hres[:, (i % tc) * ew:(i % tc) * ew + D],
                            start=st,
                            stop=sp,
                            skip_group_check=True,
                        )
                nc.gpsimd.memset(ostage[:, :D], 0.0)
            loop_cm = (tc_ctx.For_i(0, hw_loop, 1)
                       if hw_loop and variant != "peonly"
                       else contextlib.nullcontext())
            with loop_cm:
              for _rep, ch in (((r, c) for r in range(reps)
                                for c in range(nchunks))
                               if variant != "peonly" else ()):
                htile = hp.tile([P, tc * ew], hdt)
                nc.sync.dma_start(htile[:], hin[ch])
                if variant == "dmaonly":
                    continue
                for (k, w, b, diag, col, start, stop, alloc) in gsched[ch]:
                    g = ch * tc + k
                    ohtile = None
                    if variant != "nodve" and not diag:
                        # tensor_scalar: iota row packed bf16 (in0), slot id
                        # as per-partition scalar -> DVE 4x_2p perf mode
                        ohtile = op.tile([P, P], mybir.dt.bfloat16)
                        nc.vector.tensor_scalar(
                            out=ohtile[:],
                            in0=iota[:],
                            scalar1=lall[:, oix_of[g]:oix_of[g] + 1],
                            scalar2=None,
                            op0=mybir.AluOpType.is_equal,
                        )
                    if variant == "nope":
                        continue
                    oh = (const_oh[:] if (variant == "nodve" or diag)
                          else ohtile[:])
                    nw = D if variant == "halfpe" else w * ew
                    if alloc:
                        psums[b] = pp.tile([P, pw], mybir.dt.float32,
                                           name="ps")
                    psum = psums[b]
                    nc.tensor.matmul(
                        psum[:, col * D:col * D + nw],
                        lhsT=oh,
                        rhs=htile[:, k * ew:k * ew + nw],
                        start=start,
                        stop=stop,
                        skip_group_check=True,
                    )
                    if stop:
                        # DVE may read only one PSUM operand per instruction
                        nc.vector.tensor_copy(
                            out=ostage[:, b * D:(b + 1) * D],
                            in_=psum[:, :D],
                        )
                        if pw == 2 * D:
                            nc.vector.tensor_add(
                                out=ostage[:, b * D:(b + 1) * D],
                                in0=ostage[:, b * D:(b + 1) * D],
                                in1=psum[:, D:2 * D],
                            )
                        # stream completed quarters of ostage out early so
                        # the final store overlaps compute
                        if not small_out and _rep == reps - 1:
                            q = nb // 4
                            if b + 1 in (q, 2 * q, 3 * q):
                                s = (b + 1 - q) * D
                                ostore(
                                    out[:, s:(b + 1) * D],
                                    ostage[:, s:(b + 1) * D])
              if hw_loop:
                if small_out:
                    ostore(out[:], ostage[:, :D])
                else:
                    q = nb // 4
                    ostore(out[:, 3 * q * D:], ostage[:, 3 * q * D:])
            if not hw_loop:
                if small_out:
                    ostore(out[:], ostage[:, :D])
                else:
                    q = nb // 4
                    ostore(out[:, 3 * q * D:], ostage[:, 3 * q * D:])
    nc.finalize()
    _BUILD_CACHE[key] = nc
    return nc


def _host_prep(H: np.ndarray, seg: np.ndarray, ncores: int, nb: int,
               tc: int):
    """Sorted-deal sharding: segments sorted by count desc are dealt in
    groups of 128 similar-count segments to (core = g % ncores,
    local block = g // ncores, slot = position).  Within a block the slot
    counts are nearly uniform, so most tiles are identity-diagonals (the
    r-th token of slot s at partition s of tile r) needing no one-hot
    build; per-slot overflow beyond each block's diagonal depth J goes to
    dense one-hot tiles.  Per-local-block budgets (J, T_overflow) are
    shared by all cores (one static program)."""
    n, d = H.shape
    v = ncores * nb * P
    nblocks = ncores * nb
    cnt_seg = np.bincount(seg, minlength=v)
    order_seg = np.argsort(-cnt_seg, kind="stable")
    # seg order_seg[i]: group g = i // P, slot = i % P
    g_of = np.arange(v) // P
    blk_of = np.empty(v, np.int32)
    slot_of = np.empty(v, np.int32)
    blk_of[order_seg] = ((g_of % ncores) * nb + g_of // ncores).astype(np.int32)
    slot_of[order_seg] = (np.arange(v) % P).astype(np.int32)

    # per-(block, slot) counts and per-local-block budgets
    per_bs = np.zeros((nblocks, P), np.int64)
    per_bs[blk_of, slot_of] = cnt_seg
    budgets = []
    for lb in range(nb):
        M = per_bs[[c * nb + lb for c in range(ncores)]]
        best = None
        for J in range(int(M.min()), int(M.max()) + 1):
            ovf = int(np.maximum(M - J, 0).sum(axis=1).max())
            tov = -(-ovf // P)
            # <= : on equal totals prefer larger J (fewer one-hot tiles)
            if best is None or J + tov <= best[0] + best[1]:
                best = (J, tov)
        budgets.append(best)
    budgets = tuple(budgets)
    off = np.zeros(nb + 1, np.int64)
    np.cumsum(np.array([j + t for j, t in budgets]), out=off[1:])
    ntiles = int(off[-1])
    nchunks = int(math.ceil(ntiles / tc))
    rows_pad = nchunks * tc * P

    # token destinations
    key = blk_of[seg].astype(np.int64) * P + slot_of[seg]
    order = np.argsort(key, kind="stable")
    skey = key[order]
    cnt_key = np.bincount(skey, minlength=nblocks * P)
    starts_key = np.zeros(nblocks * P + 1, np.int64)
    np.cumsum(cnt_key, out=starts_key[1:])
    r = np.arange(n) - starts_key[skey]            # rank within (block, slot)
    sblk = skey // P
    cnt_blk = np.bincount(sblk, minlength=nblocks)
    starts_blk = np.zeros(nblocks + 1, np.int64)
    np.cumsum(cnt_blk, out=starts_blk[1:])
    lbs = sblk % nb
    core = sblk // nb
    J_of = np.array([b[0] for b in budgets], np.int64)[lbs]
    is_ovf = r >= J_of
    co = np.cumsum(is_ovf)
    coz = np.concatenate([[0], co])[starts_blk[sblk]]  # ovf before block
    q = co - 1 - coz                                   # ovf index in block
    tile_idx = np.where(is_ovf, off[lbs] + J_of + q // P, off[lbs] + r)
    # remap to the block-interleaved emission order used by the device
    tsched_new, old_idx = _order(budgets)
    perm = np.empty(ntiles, np.int64)
    perm[np.array(old_idx, np.int64)] = np.arange(ntiles)
    tile_idx = perm[tile_idx]
    part = np.where(is_ovf, q % P, skey % P)
    dstrow = tile_idx * P + part
    lo_val = (skey % P).astype(np.float32)

    iota = np.broadcast_to(np.arange(P), (P, P)).astype(ml_dtypes.bfloat16)
    ident = np.eye(P, dtype=ml_dtypes.bfloat16)
    in_maps = []
    for c in range(ncores):
        sel = core == c
        dst_c = dstrow[sel]
        assert dst_c.max() < rows_pad
        assert np.unique(dst_c).size == dst_c.size, "dst collision"
        rows = np.zeros((rows_pad, d), np.float32)
        lo = np.zeros(rows_pad, np.float32)
        rows[dst_c] = H[order[sel]]
        lo[dst_c] = lo_val[sel]
        if PRECISION == "e3m4":
            hl = np.clip(rows * E3_SCALE, -15.5, 15.5).astype(
                ml_dtypes.float8_e3m4).reshape(rows_pad // P, P, d)
            ew = d
        else:
            hi = rows.astype(ml_dtypes.bfloat16)
            if PRECISION == "split":
                lo_res = (rows - hi.astype(np.float32)).astype(
                    ml_dtypes.bfloat16)
                hl = np.concatenate(
                    [hi.reshape(rows_pad // P, P, d),
                     lo_res.reshape(rows_pad // P, P, d)],
                    axis=-1,
                )  # [ntiles_pad, P, 2D]
                ew = 2 * d
            else:
                hl = hi.reshape(rows_pad // P, P, d)
                ew = d
        hdev = np.ascontiguousarray(
            hl.reshape(nchunks, tc, P, ew).transpose(0, 2, 1, 3)
        ).reshape(nchunks, P, tc * ew)
        # slot-ids only for the overflow (one-hot) tiles, in schedule order
        ovf_g = np.array([i for i, (_b2, dg2) in enumerate(tsched_new)
                          if not dg2], dtype=np.int64)
        if ovf_g.size:
            lodev = np.ascontiguousarray(
                lo.reshape(nchunks * tc, P)[ovf_g].T)
        else:
            lodev = np.zeros((P, 1), np.float32)
        in_maps.append({"h": hdev, "lo": lodev, "iota": iota,
                        "ident": ident})
    outperm = blk_of.astype(np.int64) * P + slot_of  # seg -> output slot
    return in_maps, budgets, nchunks, outperm


def _unshard(results, ncores: int, nb: int, outperm: np.ndarray) -> np.ndarray:
    full = np.empty((ncores * nb * P, D), np.float32)
    for c in range(ncores):
        o = np.asarray(results[c]["out"]).astype(np.float32).reshape(P, nb, D)
        full[c * nb * P:(c + 1) * nb * P] = (
            o.transpose(1, 0, 2).reshape(nb * P, D)
        )
    if PRECISION == "e3m4":
        full *= 1.0 / E3_SCALE
    return full[outperm]


def _run(H, X_node, trace=False, trace_kwargs=None):
    H = np.ascontiguousarray(np.asarray(H, dtype=np.float32))
    seg = np.asarray(X_node).astype(np.int64)
    in_maps, budgets, nchunks, outperm = _host_prep(H, seg, NCORES, NB, TC)
    nc = _build(NB, budgets, TC, nchunks)
    kwargs = {}
    if trace:
        kwargs = dict(trace=True, trace_cores=list(range(NCORES)),
                      stitch_traces=False)
        if trace_kwargs:
            kwargs.update(trace_kwargs)
    res = run_bass_kernel_spmd(nc, in_maps, core_ids=list(range(NCORES)),
                               **kwargs)
    out = _unshard(res.results, NCORES, NB, outperm[:V])
    return out, res


def kernel(H, X_node) -> np.ndarray:
    out, _ = _run(H, X_node, trace=False)
    return out


if __name__ == "__main__":
    # tiny smoke test on hardware (all 8 cores, small V')
    rng = np.random.default_rng(0)
    n_small, v_small, nb_small, tc_small = 6000, NCORES * 2 * P, 2, 4
    Hs = rng.standard_normal((n_small, D)).astype(np.float32)
    segs = rng.integers(0, v_small, size=n_small).astype(np.int64)
    in_maps, budgets, nchunks, outperm = _host_prep(Hs, segs, NCORES, nb_small,
                                                    tc_small)
    nc = _build(nb_small, budgets, tc_small, nchunks)
    res = run_bass_kernel_spmd(nc, in_maps, core_ids=list(range(NCORES)))
    got = _unshard(res.results, NCORES, nb_small, outperm[:v_small])
    # compare against the segment-sum of the DECODED codes: isolates a
    # HW-vs-ml_dtypes encoding mismatch (e.g. fp8 subnormal flush) from
    # the intended quantization error
    if PRECISION == "e3m4":
        Hq = (np.clip(Hs * E3_SCALE, -15.5, 15.5)
              .astype(ml_dtypes.float8_e3m4).astype(np.float32) / E3_SCALE)
    elif PRECISION == "bf16":
        Hq = Hs.astype(ml_dtypes.bfloat16).astype(np.float32)
    else:
        Hq = Hs
    exp = np.zeros((v_small, D), np.float32)
    np.add.at(exp, segs, Hq)
    if PRECISION != "split":   # device stores the output as bf16
        exp = (exp * E3_SCALE if PRECISION == "e3m4" else exp)
        exp = exp.astype(ml_dtypes.bfloat16).astype(np.float32)
        exp = (exp / E3_SCALE if PRECISION == "e3m4" else exp)
    err = np.abs(got - exp).max() / max(1e-9, np.abs(exp).max())
    print(f"smoke: ntiles={sum(j+t for j,t in budgets)} nchunks={nchunks} max-rel-err={err:.3e}")
    assert err < 1e-4, "smoke test failed"
    print("SMOKE PASS")

